# revision 37
# baseline (speedup 1.0000x reference)
"""Trainium2 Bass kernel for 3D Haar wavelet transform (depthwise conv,
stride 2, kernel 2x2x2, 8-filter Haar bank per channel).

x: [2, 16, 128, 128, 128] f32  ->  y: [2, 128, 64, 64, 64] f32

Strategy (pure data parallel, bf16 end-to-end): the 32 (n, c) slabs are
split 4-per-core across 8 NeuronCores. Inputs are rounded to bf16 on the
host (halves HBM read traffic; rel-err ~2e-3, far under the 2e-2 gate) and
outputs are written bf16 and widened on the host (halves write traffic).

Per slab [d=128, h=128, w=128] the separable Haar transform runs as:
  1. TensorE: d-axis butterfly via a 128x128 bf16 matrix AND h-axis
     butterfly fused via paired accumulating matmuls over even/odd h-row
     views with +B / -B, streaming rhs columns w-deinterleaved so PSUM
     comes out (b, t, hh, wh) with the w-pairs separated.
  2. ScalarE: single flat copy evicts PSUM f32 -> SBUF bf16.
  3. VectorE: w-axis butterfly as 4 flat step-1 bf16 tensor_add/sub
     (DVE 2x_1P mode) straight into merged staging tiles laid out for
     contiguous output DMA.
Input DMAs (4 MiB, 32 KiB/partition runs) issue on SyncE/HWDGE; output
DMAs (2 MiB merged, 4 KiB/partition runs) issue on GpSimd/SWDGE so their
sem waits never stall the compute engines' sequencers. Staging is
double-buffered so the next slab's butterflies overlap the output drain.
Measured ~97-100 us/exec, at the measured ~94-100 us pure-DMA floor
(HBM-bound: 33.5 MB/core at the ~358 GB/s per-core HBM limit).
"""

import sys

if "/opt/trn_rl_repo" not in sys.path:
    sys.path.insert(0, "/opt/trn_rl_repo")

import numpy as np

N_CORES = 8
SLABS = 4          # (n, c) slabs per core
D = 128
H = 128
WID = 128
HC = 16            # h-rows per chunk
NCHUNK = H // HC   # 8 chunks per slab
FREE = HC * WID    # 2048 bf16 per partition per chunk
DH = D // 2        # 64
HH = H // 2
WH = WID // 2

CFG = dict(
    in_batch=8, x_bufs=3, cw_bufs=3, st_split=2, st_bufs=2, out_eng="gpsimd",
    in_eng="sync", merge_out=True, dma_only=False, amaj=False,
)


def _haar_weight_np() -> np.ndarray:
    lo = np.array([1.0, 1.0], dtype=np.float32) / 2
    hi = np.array([1.0, -1.0], dtype=np.float32) / 2
    filts = []
    for a in (lo, hi):
        for b in (lo, hi):
            for c in (lo, hi):
                filts.append(a[:, None, None] * b[None, :, None] * c[None, None, :])
    return np.stack(filts)


def _butterfly_lhsT(amaj: bool = True) -> np.ndarray:
    # lhsT[k, m]: matmul computes out[m, n] = sum_k lhsT[k, m] * rhs[k, n].
    # Output partition encodes (a, dh): a=0 -> d-axis low-pass sum of planes
    # (2dh, 2dh+1), a=1 -> high-pass difference. amaj: m = a*64 + dh, so in
    # the output DMA each SDMA engine's 8 partitions cover consecutive dh of
    # ONE filter plane -> each engine writes one contiguous HBM region.
    # Else m = 2*dh + a (dh-major). The full 1/8 = (1/2)^3 scale of the
    # separable transform is folded here so the h/w stages are pure add/sub.
    # Returns [128, 256]: cols 0-127 = B, cols 128-255 = -B (the h-axis
    # high-pass accumulation weight).
    b = np.zeros((128, 128), dtype=np.float32)
    f = np.float32(0.125)
    for j in range(64):
        m_lo = j if amaj else 2 * j
        m_hi = 64 + j if amaj else 2 * j + 1
        b[2 * j, m_lo] = f
        b[2 * j + 1, m_lo] = f
        b[2 * j, m_hi] = f
        b[2 * j + 1, m_hi] = -f
    return np.concatenate([b, -b], axis=1)


def build_module(n_iters: int = 1, cfg: dict | None = None):
    """Build the per-core SPMD Bass module. n_iters > 1 wraps the whole body
    in a dynamic repeat loop (used only for timing measurements)."""
    import concourse.bacc as bacc
    import concourse.mybir as mybir
    import concourse.tile as tile
    from contextlib import ExitStack

    c = dict(CFG)
    if cfg:
        c.update(cfg)
    in_batch = c["in_batch"]
    x_bufs = c["x_bufs"]
    cw_bufs = c["cw_bufs"]
    st_split = c["st_split"]
    st_bufs = c["st_bufs"]
    out_eng = c["out_eng"]
    in_eng = c["in_eng"]
    dma_only = c["dma_only"]
    amaj = c["amaj"]
    # a-major partitions can't express the 4-dim merged dst AP (only the
    # dh-major (a, bc) stride pair nests); fall back to per-(b,c) DMAs.
    merge_out = c["merge_out"] and not amaj
    chunks_per_split = NCHUNK // st_split

    fp32 = mybir.dt.float32
    bf16 = mybir.dt.bfloat16
    nc = bacc.Bacc("TRN2", target_bir_lowering=False, debug=False)

    x_d = nc.dram_tensor("x", [SLABS, D, H * WID], bf16, kind="ExternalInput")
    b_d = nc.dram_tensor("bmat", [128, 256], bf16, kind="ExternalInput")
    y_d = nc.dram_tensor("y", [SLABS, 8, DH, HH, WH], bf16, kind="ExternalOutput")

    x_ap = x_d.ap()
    y_ap = y_d.ap()

    with tile.TileContext(nc) as tc:
        with ExitStack() as ctx:
            const_pool = ctx.enter_context(tc.tile_pool(name="const", bufs=1))
            x_pool = ctx.enter_context(tc.tile_pool(name="xin", bufs=x_bufs))
            c_pool = ctx.enter_context(tc.tile_pool(name="cpy", bufs=cw_bufs))
            st_pool = ctx.enter_context(tc.tile_pool(name="stage", bufs=st_bufs))
            psum_pool = ctx.enter_context(
                tc.tile_pool(name="psum", bufs=2, space="PSUM")
            )

            bt = const_pool.tile([128, 256], bf16)
            nc.sync.dma_start(bt[:], b_d.ap()[:])
            bpos = bt[:, 0:128]   # +B
            bneg = bt[:, 128:256]  # -B

            def dma_only_body(_i=None):
                stf = HH * WH // st_split
                stz = {}
                for hf in range(st_split):
                    n_t = 1 if merge_out else 4
                    for j in range(n_t):
                        t = st_pool.tile(
                            [128, 4 * stf // n_t], bf16, tag=f"stz{j}_{hf}",
                            name=f"stz{j}_{hf}",
                        )
                        stz[j, hf] = t
                        nc.vector.memset(t[:], 0.0)
                oeng = getattr(nc, out_eng)
                for s in range(SLABS):
                    for qb in range(NCHUNK // in_batch):
                        xtb = x_pool.tile(
                            [128, FREE * in_batch], bf16, tag="xt", name="xt"
                        )
                        getattr(nc, in_eng).dma_start(
                            xtb[:],
                            x_ap[s][
                                :, qb * FREE * in_batch : (qb + 1) * FREE * in_batch
                            ],
                        )
                    yvm = y_ap[s].rearrange(
                        "(a b c) dh (hf hh) wh -> hf a dh (b c) (hh wh)"
                        if amaj else
                        "(a b c) dh (hf hh) wh -> hf dh a (b c) (hh wh)",
                        a=2, b=2, c=2, hf=st_split,
                    )
                    yvs = y_ap[s].rearrange(
                        "(a b c) dh (hf hh) wh -> (b c) hf a dh (hh wh)"
                        if amaj else
                        "(a b c) dh (hf hh) wh -> (b c) hf dh a (hh wh)",
                        a=2, b=2, c=2, hf=st_split,
                    )
                    for hf in range(st_split):
                        if merge_out:
                            oeng.dma_start(yvm[hf], stz[0, hf][:])
                        else:
                            for bc in range(4):
                                oeng.dma_start(yvs[bc][hf], stz[bc, hf][:])

            def body(_i=None):
                for s in range(SLABS):
                    # staging tiles: one [128, 4*stf] tile per hh-split
                    # (free = (bc, hh, wh)) when merge_out, else one
                    # [128, stf] tile per ((b, c), hh-split).
                    stf = HH * WH // st_split
                    sts = {}
                    stm = {}
                    for hf in range(st_split):
                        if merge_out:
                            t = st_pool.tile(
                                [128, 4 * stf], bf16, tag=f"st_{hf}",
                                name=f"st_{hf}",
                            )
                            stm[hf] = t
                            for bc in range(4):
                                sts[bc, hf] = t[:, bc * stf : (bc + 1) * stf]
                        else:
                            for bc in range(4):
                                sts[bc, hf] = st_pool.tile(
                                    [128, stf], bf16, tag=f"st{bc}_{hf}",
                                    name=f"st{bc}_{hf}",
                                )
                    # staging partition p = a*64 + dh (amaj) or 2*dh + a
                    yvs = y_ap[s].rearrange(
                        "(a b c) dh (hf hh) wh -> (b c) hf a dh (hh wh)"
                        if amaj else
                        "(a b c) dh (hf hh) wh -> (b c) hf dh a (hh wh)",
                        a=2, b=2, c=2, hf=st_split,
                    )
                    yvm = y_ap[s].rearrange(
                        "(a b c) dh (hf hh) wh -> hf a dh (b c) (hh wh)"
                        if amaj else
                        "(a b c) dh (hf hh) wh -> hf dh a (b c) (hh wh)",
                        a=2, b=2, c=2, hf=st_split,
                    )
                    xts = {}
                    for q in range(NCHUNK):
                        qb, qo = divmod(q, in_batch)
                        if qo == 0:
                            xtb = x_pool.tile(
                                [128, FREE * in_batch], bf16, tag="xt", name="xt"
                            )
                            xts[qb] = xtb
                            ie = (
                                ("sync", "vector")[qb % 2] if in_eng == "alt"
                                else in_eng
                            )
                            getattr(nc, ie).dma_start(
                                xtb[:],
                                x_ap[s][
                                    :,
                                    qb * FREE * in_batch : (qb + 1) * FREE * in_batch,
                                ],
                            )
                        xt = xts[qb][:, qo * FREE : (qo + 1) * FREE]
                        # chunk free index = h_local*128 + w, h_local = 2*hh+sp,
                        # w = 2*wh + t
                        xv = xt.rearrange(
                            "p (hh sp wh t) -> p sp t hh wh", sp=2, t=2, wh=WH
                        )
                        pt = psum_pool.tile([128, FREE], fp32, tag="pt")
                        # PSUM free layout (b, t, hh, wh): four 512-col banks
                        pv = pt.rearrange("p (b t f) -> p b t f", b=2, t=2)
                        for hb in range(2):
                            for wt in range(2):
                                dst = pv[:, hb, wt]
                                nc.tensor.matmul(
                                    dst, bpos, xv[:, 0, wt],
                                    start=True, stop=False,
                                )
                                nc.tensor.matmul(
                                    dst, bpos if hb == 0 else bneg, xv[:, 1, wt],
                                    start=False, stop=True,
                                )
                        # evict PSUM -> SBUF bf16 on the (otherwise idle)
                        # scalar engine; DVE tensor_tensor may read only one
                        # PSUM operand and the w-butterfly needs two.
                        ct = c_pool.tile([128, FREE], bf16, tag="ct", name="ct")
                        nc.scalar.copy(ct[:], pt[:])
                        cv = ct.rearrange("p (b t f) -> p b t f", b=2, t=2)
                        hf, ql = divmod(q, chunks_per_split)
                        of0 = ql * (HC // 2) * WH
                        for hb in range(2):
                            nc.vector.tensor_add(
                                sts[hb * 2 + 0, hf][:, of0 : of0 + 512],
                                cv[:, hb, 0],
                                cv[:, hb, 1],
                            )
                            nc.vector.tensor_sub(
                                sts[hb * 2 + 1, hf][:, of0 : of0 + 512],
                                cv[:, hb, 0],
                                cv[:, hb, 1],
                            )
                        if ql == chunks_per_split - 1:
                            # this hh-split of staging is complete
                            oeng = getattr(
                                nc,
                                ("gpsimd", "scalar")[hf % 2]
                                if out_eng == "alt" else out_eng,
                            )
                            if merge_out:
                                oeng.dma_start(yvm[hf], stm[hf][:])
                            else:
                                for bc in range(4):
                                    oeng.dma_start(yvs[bc][hf], sts[bc, hf][:])

            fn = dma_only_body if dma_only else body
            if n_iters == 1:
                fn()
            else:
                with tc.For_i(0, n_iters, 1) as i:
                    fn(i)

    nc.compile()
    nc._haar_cfg = c
    return nc


_CACHED_NC = None


def _get_nc():
    global _CACHED_NC
    if _CACHED_NC is None:
        _CACHED_NC = build_module(1)
    return _CACHED_NC


def make_in_maps(x: np.ndarray, cfg: dict | None = None) -> list:
    """Shard + bf16-convert the full f32 input for the 8 cores."""
    import ml_dtypes

    c = dict(CFG)
    if cfg:
        c.update(cfg)
    xf = np.ascontiguousarray(x.reshape(32, D, H * WID)).astype(ml_dtypes.bfloat16)
    bmat = _butterfly_lhsT(c["amaj"]).astype(ml_dtypes.bfloat16)
    return [
        {"x": xf[SLABS * k : SLABS * (k + 1)], "bmat": bmat} for k in range(N_CORES)
    ]


def _numpy_fallback(x: np.ndarray, w: np.ndarray) -> np.ndarray:
    n, c, d, h, wd = x.shape
    xb = x.reshape(n, c, d // 2, 2, h // 2, 2, wd // 2, 2)
    y = np.einsum("ncdihjwk,oijk->ncodhw", xb, w)
    return y.reshape(n, c * 8, d // 2, h // 2, wd // 2).astype(x.dtype)


def kernel(x: np.ndarray, W: np.ndarray) -> np.ndarray:
    from concourse import bass_utils

    x = np.asarray(x)
    W = np.asarray(W)
    if not np.allclose(W, _haar_weight_np(), rtol=0, atol=1e-12):
        # The butterfly factorization is specialized to the exact Haar bank.
        return _numpy_fallback(x, W)

    n, c, d, h, wd = x.shape
    assert (n, c, d, h, wd) == (2, 16, 128, 128, 128), x.shape

    nc = _get_nc()
    in_maps = make_in_maps(x)
    res = bass_utils.run_bass_kernel_spmd(nc, in_maps, core_ids=list(range(N_CORES)))
    y = np.stack([np.asarray(res.results[k]["y"]) for k in range(N_CORES)])
    # [8, 4, 8, dh, hh, wh] -> [2, 128, dh, hh, wh], widen bf16 -> f32
    return y.reshape(2, 128, DH, HH, WH).astype(np.float32)


if __name__ == "__main__":
    rng = np.random.default_rng(0)
    x = rng.standard_normal((2, 16, 128, 128, 128), dtype=np.float32)
    w = _haar_weight_np()
    out = kernel(x, w)
    exp = _numpy_fallback(x, w)
    err = np.abs(out - exp).max() / np.abs(exp).max()
    print("rel err vs numpy:", err)
